# revision 1
# baseline (speedup 1.0000x reference)
"""Trainium2 Bass kernel for nn_HOANLayer (GAT-style bilinear attention layer).

Math:
  xw_s = x_source @ w_source; xw_t = x_target @ w_target          [N, d]
  e_ij = lrelu(s1_i + t2_j), f_ji = lrelu(t1_j + s2_i)            [N, N]
  att_s = softmax_rows(mask ? e : -1e13)
  att_t = softmax_rows(mask.T ? f : -1e13)
  out_s = elu(att_s @ xw_s + bias_s); out_t = elu(att_t @ xw_t + bias_t)

Key identities exploited on device:
  exp(lrelu(z)) = max(exp(z), exp(0.01 z))        (lrelu(z) = max(z, 0.01 z), exp monotone)
  exp(s1_i + t2_j) = exp(s1_i) * exp(t2_j)        (rank-1 separability)
so the masked softmax numerator is
  n_ij = m_ij * max(exp(s1_i)exp(t2_j), exp(.01 s1_i)exp(.01 t2_j))
       = [m_ij * exp(.01 s1_i)] * max(exp(.99 s1_i)*exp(t2_j), exp(.01 t2_j))
The bracket is folded into the mask on the host; the max() term is ONE fused
VectorE tensor_scalar (op0=mult, op1=max with per-partition scalar vectors) on a
broadcast tile, so no transcendentals run on device. Row sums for the softmax
come for free from a ones-column appended to xw in the output matmul.

Sharding: row-block over 8 cores. Core c computes update_source rows
[c*1024,(c+1)*1024) (layout [j-partitions, i-free], mask = adjacency[I,:].T) and
update_target rows [c*1024,(c+1)*1024) (layout [i-partitions, j-free], mask =
adjacency[:, J]). Division by row sums, elu, bias, transpose: host.
"""

import numpy as np
import ml_dtypes

BF16 = ml_dtypes.bfloat16

N = 8192
D = 64
M65 = D + 1
NCORES = 8
R = N // NCORES  # 1024 rows per core
P = 128
SLOPE = 0.01


_CACHE = {}


def _build_program(n_rows, blk, num_devices):
    """Build + compile the SPMD Bass program.

    n_rows: contraction length (full N), blk: per-core row-block width (free dim).
    """
    from contextlib import ExitStack

    import concourse.bass as bass
    import concourse.bacc as bacc
    import concourse.tile as tile
    from concourse import mybir

    f32 = mybir.dt.float32
    bf16 = mybir.dt.bfloat16
    kt = n_rows // P
    mm_chunk = 512

    nc = bacc.Bacc(
        "TRN2",
        target_bir_lowering=False,
        debug=False,
        num_devices=num_devices,
    )

    d_mask_e = nc.dram_tensor("mask_e", [n_rows, blk], bf16, kind="ExternalInput").ap()
    d_mask_f = nc.dram_tensor("mask_f", [n_rows, blk], bf16, kind="ExternalInput").ap()
    d_brd_e = nc.dram_tensor("brd_e", [P, blk], bf16, kind="ExternalInput").ap()
    d_brd_f = nc.dram_tensor("brd_f", [P, blk], bf16, kind="ExternalInput").ap()
    d_tabB_e = nc.dram_tensor("tabB_e", [P, kt], f32, kind="ExternalInput").ap()
    d_tabb_e = nc.dram_tensor("tabb_e", [P, kt], f32, kind="ExternalInput").ap()
    d_tabS_f = nc.dram_tensor("tabS_f", [P, kt], f32, kind="ExternalInput").ap()
    d_tabs_f = nc.dram_tensor("tabs_f", [P, kt], f32, kind="ExternalInput").ap()
    d_xwp_s = nc.dram_tensor("xwp_s", [P, kt * M65], bf16, kind="ExternalInput").ap()
    d_xwp_t = nc.dram_tensor("xwp_t", [P, kt * M65], bf16, kind="ExternalInput").ap()
    d_out_e = nc.dram_tensor("out_e", [M65, blk], f32, kind="ExternalOutput").ap()
    d_out_f = nc.dram_tensor("out_f", [M65, blk], f32, kind="ExternalOutput").ap()

    mult = mybir.AluOpType.mult
    maxop = mybir.AluOpType.max

    with tile.TileContext(nc) as tc:
        with ExitStack() as ctx:
            cpool = ctx.enter_context(tc.tile_pool(name="consts", bufs=1))
            mpool = ctx.enter_context(tc.tile_pool(name="masks", bufs=4))
            wpool = ctx.enter_context(tc.tile_pool(name="work", bufs=3))
            opool = ctx.enter_context(tc.tile_pool(name="outs", bufs=1))
            ppool = ctx.enter_context(
                tc.tile_pool(name="psum", bufs=1, space=bass.MemorySpace.PSUM)
            )

            dma = nc.default_dma_engine.dma_start

            brd_e = cpool.tile([P, blk], bf16)
            dma(brd_e[:], d_brd_e[:])
            brd_f = cpool.tile([P, blk], bf16)
            dma(brd_f[:], d_brd_f[:])
            tabB_e = cpool.tile([P, kt], f32)
            dma(tabB_e[:], d_tabB_e[:])
            tabb_e = cpool.tile([P, kt], f32)
            dma(tabb_e[:], d_tabb_e[:])
            tabS_f = cpool.tile([P, kt], f32)
            dma(tabS_f[:], d_tabS_f[:])
            tabs_f = cpool.tile([P, kt], f32)
            dma(tabs_f[:], d_tabs_f[:])
            xwp_s = cpool.tile([P, kt * M65], bf16)
            dma(xwp_s[:], d_xwp_s[:])
            xwp_t = cpool.tile([P, kt * M65], bf16)
            dma(xwp_t[:], d_xwp_t[:])

            ps_e = ppool.tile([M65, blk], f32)
            ps_f = ppool.tile([M65, blk], f32)

            nch = (blk + mm_chunk - 1) // mm_chunk

            for k in range(kt):
                rows = slice(P * k, P * (k + 1))
                wcol = slice(M65 * k, M65 * (k + 1))

                me = mpool.tile([P, blk], bf16, tag="me")
                dma(me[:], d_mask_e[rows, :])
                pe = wpool.tile([P, blk], bf16, tag="pe")
                nc.vector.tensor_scalar(
                    out=pe[:],
                    in0=brd_e[:],
                    scalar1=tabB_e[:, k : k + 1],
                    scalar2=tabb_e[:, k : k + 1],
                    op0=mult,
                    op1=maxop,
                )
                ne = wpool.tile([P, blk], bf16, tag="ne")
                nc.vector.tensor_tensor(ne[:], pe[:], me[:], op=mult)
                for c in range(nch):
                    cs = slice(c * mm_chunk, min((c + 1) * mm_chunk, blk))
                    nc.tensor.matmul(
                        ps_e[:, cs],
                        xwp_s[:, wcol],
                        ne[:, cs],
                        start=(k == 0),
                        stop=(k == kt - 1),
                    )

                mf = mpool.tile([P, blk], bf16, tag="mf")
                dma(mf[:], d_mask_f[rows, :])
                pf = wpool.tile([P, blk], bf16, tag="pf")
                nc.vector.tensor_scalar(
                    out=pf[:],
                    in0=brd_f[:],
                    scalar1=tabS_f[:, k : k + 1],
                    scalar2=tabs_f[:, k : k + 1],
                    op0=mult,
                    op1=maxop,
                )
                nf = wpool.tile([P, blk], bf16, tag="nf")
                nc.vector.tensor_tensor(nf[:], pf[:], mf[:], op=mult)
                for c in range(nch):
                    cs = slice(c * mm_chunk, min((c + 1) * mm_chunk, blk))
                    nc.tensor.matmul(
                        ps_f[:, cs],
                        xwp_t[:, wcol],
                        nf[:, cs],
                        start=(k == 0),
                        stop=(k == kt - 1),
                    )

            oe = opool.tile([M65, blk], f32)
            nc.scalar.copy(oe[:], ps_e[:])
            dma(d_out_e[:], oe[:])
            of = opool.tile([M65, blk], f32)
            nc.scalar.copy(of[:], ps_f[:])
            dma(d_out_f[:], of[:])

    nc.compile()
    return nc


def _get_program():
    key = (N, R, NCORES)
    if key not in _CACHE:
        _CACHE[key] = _build_program(N, R, NCORES)
    return _CACHE[key]


def _host_prep(x_source, x_target, adjacency, w_source, w_target, a):
    """All the small dense algebra + mask scaling, in numpy f32."""
    f = np.float32
    xw_s = x_source.astype(f) @ w_source.astype(f)  # [N, D]
    xw_t = x_target.astype(f) @ w_target.astype(f)
    a1 = a[:D, 0].astype(f)
    a2 = a[D:, 0].astype(f)
    s1 = xw_s @ a1
    t2 = xw_t @ a2
    t1 = xw_t @ a1
    s2 = xw_s @ a2

    kt = N // P
    ones = np.ones((N, 1), f)
    # [K, M] stationary layout packed as [128, kt*65]: tile k at cols [65k, 65k+65)
    xwp_s = (
        np.concatenate([xw_s, ones], axis=1)
        .reshape(kt, P, M65)
        .transpose(1, 0, 2)
        .reshape(P, kt * M65)
        .astype(BF16)
    )
    xwp_t = (
        np.concatenate([xw_t, ones], axis=1)
        .reshape(kt, P, M65)
        .transpose(1, 0, 2)
        .reshape(P, kt * M65)
        .astype(BF16)
    )

    # per-partition scalar tables [128, kt]: col k = vec[128k : 128k+128]
    tabB_e = np.exp(t2).reshape(kt, P).T.astype(f).copy()
    tabb_e = np.exp(SLOPE * t2).reshape(kt, P).T.astype(f).copy()
    tabS_f = np.exp(s2).reshape(kt, P).T.astype(f).copy()
    tabs_f = np.exp(SLOPE * s2).reshape(kt, P).T.astype(f).copy()

    brdv_e = np.exp((1.0 - SLOPE) * s1).astype(f)  # free-dim vector, sliced per core
    brdv_f = np.exp((1.0 - SLOPE) * t1).astype(f)

    # Host-folded negative-branch factors into the masks.
    adj_f = adjacency.astype(f)
    mask_e_all = (adj_f * np.exp(SLOPE * s1)[:, None]).T.astype(BF16)  # [N(j), N(i)]
    mask_f_all = (adj_f * np.exp(SLOPE * t1)[None, :]).astype(BF16)  # [N(i), N(j)]
    del adj_f

    return {
        "xw_s": xw_s,
        "xw_t": xw_t,
        "xwp_s": xwp_s,
        "xwp_t": xwp_t,
        "tabB_e": tabB_e,
        "tabb_e": tabb_e,
        "tabS_f": tabS_f,
        "tabs_f": tabs_f,
        "brdv_e": brdv_e,
        "brdv_f": brdv_f,
        "mask_e_all": mask_e_all,
        "mask_f_all": mask_f_all,
    }


def _core_inputs(prep, c):
    sl = slice(c * R, (c + 1) * R)
    return {
        "mask_e": np.ascontiguousarray(prep["mask_e_all"][:, sl]),
        "mask_f": np.ascontiguousarray(prep["mask_f_all"][:, sl]),
        "brd_e": np.broadcast_to(prep["brdv_e"][sl].astype(BF16), (P, R)).copy(),
        "brd_f": np.broadcast_to(prep["brdv_f"][sl].astype(BF16), (P, R)).copy(),
        "tabB_e": prep["tabB_e"],
        "tabb_e": prep["tabb_e"],
        "tabS_f": prep["tabS_f"],
        "tabs_f": prep["tabs_f"],
        "xwp_s": prep["xwp_s"],
        "xwp_t": prep["xwp_t"],
    }


def _elu(x):
    return np.where(x > 0, x, np.expm1(np.minimum(x, 0.0), dtype=np.float32)).astype(
        np.float32
    )


def run(inputs, trace=False):
    """Run the kernel; returns ((update_source, update_target), BassKernelResults)."""
    from concourse import bass_utils

    prep = _host_prep(
        inputs["x_source"],
        inputs["x_target"],
        inputs["adjacency"],
        inputs["w_source"],
        inputs["w_target"],
        inputs["a"],
    )
    nc = _get_program()
    in_maps = [_core_inputs(prep, c) for c in range(NCORES)]
    res = bass_utils.run_bass_kernel_spmd(
        nc, in_maps, list(range(NCORES)), trace=trace
    )

    bias_s = inputs["bias_source"].astype(np.float32)
    bias_t = inputs["bias_target"].astype(np.float32)
    us = np.empty((N, D), np.float32)
    ut = np.empty((N, D), np.float32)
    for c in range(NCORES):
        sl = slice(c * R, (c + 1) * R)
        oe = res.results[c]["out_e"]  # [65, R] f32
        of = res.results[c]["out_f"]
        us[sl] = _elu(oe[:D].T / oe[D][:, None] + bias_s[None, :])
        ut[sl] = _elu(of[:D].T / of[D][:, None] + bias_t[None, :])
    return (us, ut), res


def kernel(**inputs):
    (us, ut), _ = run(inputs, trace=False)
    return (us, ut)
